# revision 14
# baseline (speedup 1.0000x reference)
"""Trainium2 Bass kernel for nn_CrossModalGatedBottleneckAttention (v3).

Contract: kernel(**inputs) takes the FULL unsharded inputs (as produced by
the problem's setup_inputs) and returns the full [16, 768, 512] output.

Strategy: data parallelism over batch B=16 across 8 NeuronCores (2 batches
per core). v3 over v2: fp8(e4m3)+DoubleRow matmuls for the error-tolerant
layers (P1 projections, bottleneck attention logits, av_up/av_bj, gated
fusion f/m, prologue) — 2x PE throughput on those. The error-sensitive
tail (QKV, MHA, out-proj) and av_ib stay fp16. A_up/A_bj attention weights
are stored x16 in fp8 to stay out of e4m3 subnormals; the 1/32 (incl. the
0.5 a_ij average) is folded into the P4 PSUM drain.
"""
import sys as _sys
for _p in ("/opt/trn_rl_repo",):
    if _p not in _sys.path:
        _sys.path.insert(0, _p)

import numpy as np
import concourse.bass as bass
import concourse.mybir as mybir
import concourse.tile as tile
from concourse.bass_utils import run_bass_kernel_spmd
from concourse.masks import make_identity

# ---------------------------------------------------------------------------
# Workaround for walrus sync-wait encoding limits: several instruction
# encodings in this neuronxcc build reject more than one sem-wait per
# instruction ("Too many sync wait commands"). After Tile scheduling, move
# all but one wait of each instruction onto same-engine NoOps inserted just
# before it. An engine blocks on each wait in order, so semantics are
# preserved.
_wsplit_ctr = [0]


def _split_waits(nc, max_waits=1):
    n_split = 0
    for f in nc.m.functions:
        for blk in f.blocks:
            insts = blk.instructions
            new_list = []
            changed = False
            for inst in insts:
                si = inst.sync_info
                if si is not None and si.on_wait and len(si.on_wait) > max_waits:
                    waits = list(si.on_wait)
                    extra, keep = waits[:-max_waits], waits[-max_waits:]
                    for w in extra:
                        _wsplit_ctr[0] += 1
                        nop = mybir.InstNoOp(
                            name=f"I-wsplit-{_wsplit_ctr[0]}", ins=[], outs=[])
                        nop.engine = inst.engine
                        nop.sync_info = mybir.SyncInfo(on_wait=[w], on_update=[])
                        new_list.append(nop)
                        n_split += 1
                    inst.sync_info = mybir.SyncInfo(
                        on_wait=keep, on_update=list(si.on_update or []))
                    changed = True
                new_list.append(inst)
            if changed:
                insts.clear()
                insts.extend(new_list)
    return n_split


# ---------------------------------------------------------------------------
F32 = mybir.dt.float32
F16 = mybir.dt.float16
F8 = mybir.dt.float8e4
AF = mybir.ActivationFunctionType
ALU = mybir.AluOpType
DR = mybir.MatmulPerfMode.DoubleRow

F = 512
N = 768
NB = 256
NHEADS = 8
HD = 64
KT = F // 128            # 4 k-tiles over feature dim
KP = KT // 2             # 2 DoubleRow k-pair tiles
NT = N // 128            # 6 tiles over tokens
NBT = NB // 128          # 2 tiles over bottleneck tokens
SCALE = float(F) ** -0.5
MHA_SCALE = float(HD) ** -0.5
B_LOC = 2                # batches per core
A_SC = 16.0              # fp8 storage scale for A_up/A_bj attention weights


def mm_acc(nc, psum_ap, pairs):
    n = len(pairs)
    for i, (l, r) in enumerate(pairs):
        nc.tensor.matmul(psum_ap, l, r, start=(i == 0), stop=(i == n - 1))


def dcopy(nc, i, out, in_):
    """PSUM drain copy, alternating DVE / ACT (GpSimd cannot touch PSUM)."""
    if i % 2 == 0:
        nc.vector.tensor_copy(out, in_)
    else:
        nc.scalar.copy(out, in_)


def dscale(nc, i, out, in_, c):
    """PSUM drain with scalar multiply, alternating DVE / ACT."""
    if i % 2 == 0:
        nc.vector.tensor_scalar_mul(out, in_, c)
    else:
        nc.scalar.mul(out, in_, c)


def dr_acc(nc, ps_cols, pairs):
    """fp8 DoubleRow accumulation into a [128, n] PSUM region.

    pairs: list of (lhsT [128, 2, 128], rhs [128, 2, n]) fp8 APs; each
    matmul contracts both k-planes (256 rows) at 2 fp8 weights per PE cell.
    """
    npair = len(pairs)
    for i, (l, r) in enumerate(pairs):
        nc.tensor.matmul(ps_cols, l, r, start=(i == 0),
                         stop=(i == npair - 1), perf_mode=DR)


def build(nc: bass.Bass, repeat: int = 1):
    dram = {}

    def din(name, shape, dt=F16):
        dram[name] = nc.declare_dram_parameter(name, list(shape), dt,
                                               isOutput=False)
        return dram[name]

    for name, shape, dt in [
            ("x1t", [B_LOC, F, N], F16),
            ("x1t8", [B_LOC, F, N], F8), ("x2t8", [B_LOC, F, N], F8),
            ("zbt8", [F, NB], F8),
            ("wkv_i8", [F, 2 * F], F8), ("wq_j8", [F, F], F8),
            ("wqkv_b8", [F, 3 * F], F8),
            ("w_f8", [2 * F, F], F8), ("w_m8", [F, F], F8),
            ("w_qkv", [F, 3 * F], F16), ("w_proj", [F, F], F16)]:
        din(name, shape, dt)
    din("b_f", [F], F32)
    din("b_m", [F], F32)
    din("bproj_rep", [128, F], F32)
    out = nc.declare_dram_parameter("out", [B_LOC, N, F], F16, isOutput=True)

    with tile.TileContext(nc) as tc:
        if repeat == 1:
            _body(nc, tc, dram, out)
        else:
            with tc.For_i(0, repeat, 1,
                          hint_engines=(mybir.EngineType.PE,
                                        mybir.EngineType.DVE),
                          staggered_reset=True):
                _body(nc, tc, dram, out)
    return nc


def _wview(ap):
    # [Fin, Fout] dram -> [128, Fin//128, Fout] partition-tiled view
    return ap[:, :].rearrange("(k p) o -> p k o", p=128)


def _xview(ap):
    # [F, N] dram -> [128, KT, N]
    return ap.rearrange("(k p) n -> p k n", p=128)


def _body(nc, tc, dram, out):
    import contextlib
    with contextlib.ExitStack() as ctx:
        consts = ctx.enter_context(tc.tile_pool(name="consts", bufs=1))
        wts = ctx.enter_context(tc.tile_pool(name="wts", bufs=1))
        acts = ctx.enter_context(tc.tile_pool(name="acts", bufs=1))
        smalls = ctx.enter_context(tc.tile_pool(name="smalls", bufs=1))
        pp_st = ctx.enter_context(tc.tile_pool(name="pp_st", bufs=1, space="PSUM"))
        pp_mm = ctx.enter_context(tc.tile_pool(name="pp_mm", bufs=1, space="PSUM"))
        pp_h = ctx.enter_context(tc.tile_pool(name="pp_h", bufs=1, space="PSUM"))
        _emit(nc, tc, dram, out, consts, wts, acts, smalls, pp_st, pp_mm, pp_h)


def _small_attention(nc, smalls, pp_mm, pp_st, ident, lhs, rhs_fm, A_T, p):
    """A = A_SC * softmax(lhs.T @ rhs_fm * SCALE, axis=-1) written to A_T
    transposed [NB, N] fp8. lhs, rhs_fm fp8. Generator: yields at chunk
    boundaries."""
    E = smalls.tile([128, NT, NB], F16, tag=f"esm{p}")
    for nt in range(NT):
        ps = pp_mm.tile([128, 512], F32, tag="mm", bufs=3)
        dr_acc(nc, ps[:, :NB],
               [(lhs[:, 2 * kp:2 * kp + 2, nt * 128:(nt + 1) * 128],
                 rhs_fm[:, 2 * kp:2 * kp + 2, :]) for kp in range(KP)])
        srs = smalls.tile([128, 2], F32, tag=f"srs{p}", bufs=4)
        nc.scalar.activation(out=E[:, nt, :], in_=ps[:, :NB], func=AF.Exp,
                             scale=SCALE, accum_out=srs[:, 0:1])
        nc.vector.reciprocal(srs[:, 1:2], srs[:, 0:1])
        nc.gpsimd.tensor_scalar(out=E[:, nt, :], in0=E[:, nt, :],
                                scalar1=srs[:, 1:2], scalar2=A_SC,
                                op0=ALU.mult, op1=ALU.mult)
        if nt % 3 == 2:
            yield
    # transposes: per mt, 6 [128,128] blocks -> one PSUM bank -> wide copy
    for mt in range(NBT):
        pt = pp_mm.tile([128, 1024], F16, tag="mm", bufs=3, name="tr_sm")
        for j in range(NT):
            nc.tensor.transpose(pt[:, j * 128:(j + 1) * 128],
                                E[:, j, mt * 128:(mt + 1) * 128], ident[:, :])
        nc.vector.tensor_copy(A_T[:, mt, :], pt[:, :NT * 128])
        yield


def _emit(nc, tc, dram, out, consts, wts, acts, smalls, pp_st, pp_mm, pp_h):
    # ---- constants -------------------------------------------------
    ident = consts.tile([128, 128], F16)
    make_identity(nc, ident)
    bf_c = consts.tile([128, KT], F32)
    nc.sync.dma_start(out=bf_c, in_=dram["b_f"][:].rearrange("(k p) -> p k", p=128))
    bm_c = consts.tile([128, KT], F32)
    nc.sync.dma_start(out=bm_c, in_=dram["b_m"][:].rearrange("(k p) -> p k", p=128))
    bproj_s = consts.tile([128, F], F32)
    nc.sync.dma_start(out=bproj_s, in_=dram["bproj_rep"][:, :])
    zrow = consts.tile([1, 390], F16)
    nc.vector.memset(zrow[:, :], 0.0)
    ones_h = consts.tile([1, 128], F16)
    nc.vector.memset(ones_h[:, :], 1.0)

    # ---- prologue: bottleneck projections (batch independent) ------
    # DMA issue order = queue order: first the tensors feeding the first PE
    # work (zbt, wqkv_b), then both batches' inputs so batch 1's loads do
    # not queue behind batch 0's output stores, then weights by first use.
    # split the wqkv_b load so the first prologue matmuls (q_b, needs cols
    # 0:F) can start before the k/v columns arrive
    wqkvb_s = wts.tile([128, KT, 3 * F], F8, tag="wqkvb")
    nc.sync.dma_start(out=wqkvb_s[:, :, 0:F], in_=_wview(dram["wqkv_b8"])[:, :, 0:F])
    zbt_s = smalls.tile([128, KT, NB], F8, tag="zbt")
    nc.sync.dma_start(out=zbt_s, in_=_xview(dram["zbt8"][:, :]))
    nc.sync.dma_start(out=wqkvb_s[:, :, F:3 * F],
                      in_=_wview(dram["wqkv_b8"])[:, :, F:3 * F])
    xs = []
    for b in range(B_LOC):
        z_it = acts.tile([128, KT, N], F16, tag=f"z_it{b % 2}", name="z_it")
        z_it8 = acts.tile([128, KT, N], F8, tag=f"z_it8{b % 2}", name="z_it8")
        z_jt8 = acts.tile([128, KT, N], F8, tag=f"z_jt8{b % 2}", name="z_jt8")
        xs.append((z_it, z_it8, z_jt8))
    nc.sync.dma_start(out=xs[0][1], in_=_xview(dram["x1t8"][0]))
    nc.sync.dma_start(out=xs[0][2], in_=_xview(dram["x2t8"][0]))
    wkvi_s = wts.tile([128, KT, 2 * F], F8, tag="wkvi")
    nc.sync.dma_start(out=wkvi_s, in_=_wview(dram["wkv_i8"]))
    wqj_s = wts.tile([128, KT, F], F8, tag="wqj")
    nc.sync.dma_start(out=wqj_s, in_=_wview(dram["wq_j8"]))
    nc.sync.dma_start(out=xs[0][0], in_=_xview(dram["x1t"][0]))
    nc.sync.dma_start(out=xs[1][1], in_=_xview(dram["x1t8"][1]))
    nc.sync.dma_start(out=xs[1][2], in_=_xview(dram["x2t8"][1]))
    nc.sync.dma_start(out=xs[1][0], in_=_xview(dram["x1t"][1]))
    wf_s = wts.tile([128, 2 * KT, F], F8, tag="wf")
    nc.sync.dma_start(out=wf_s, in_=_wview(dram["w_f8"]))
    wm_s = wts.tile([128, KT, F], F8, tag="wm")
    nc.sync.dma_start(out=wm_s, in_=_wview(dram["w_m8"]))
    wqkv_s = wts.tile([128, KT, 3 * F], F16, tag="wqkv")
    nc.sync.dma_start(out=wqkv_s, in_=_wview(dram["w_qkv"]))
    wproj_s = wts.tile([128, KT, F], F16, tag="wproj")
    nc.sync.dma_start(out=wproj_s, in_=_wview(dram["w_proj"]))

    q_bT = consts.tile([128, KT, NB], F8)
    k_bT = consts.tile([128, KT, NB], F8)
    for dst, co in ((q_bT, 0), (k_bT, F)):
        for mt in range(KT):
            ps = pp_mm.tile([128, 512], F32, tag="mm", bufs=3)
            dr_acc(nc, ps[:, :NB],
                   [(wqkvb_s[:, 2 * kp:2 * kp + 2,
                             co + mt * 128:co + (mt + 1) * 128],
                     zbt_s[:, 2 * kp:2 * kp + 2, :]) for kp in range(KP)])
            dscale(nc, mt, dst[:, mt, :], ps[:, :NB], 0.2)
    v_b = consts.tile([128, NBT, F], F8)
    for mt in range(NBT):
        ps = pp_mm.tile([128, 512], F32, tag="mm", bufs=3)
        dr_acc(nc, ps[:, :],
               [(zbt_s[:, 2 * kp:2 * kp + 2, mt * 128:(mt + 1) * 128],
                 wqkvb_s[:, 2 * kp:2 * kp + 2, 2 * F:3 * F])
                for kp in range(KP)])
        dscale(nc, mt + 1, v_b[:, mt, :], ps[:, :], 0.2)

    # MHA V layout (per batch parity): per token-tile, 8 heads x (64 V cols
    # + 1 ones col for the softmax denominator)
    Vps = []
    for p in range(2):
        V_plus = smalls.tile([128, NT, NHEADS * (HD + 1)], F16, tag=f"vp{p}",
                             name="V_plus")
        Vp_h = V_plus.rearrange("p n (h c) -> p n h c", c=HD + 1)
        nc.vector.memset(Vp_h[:, :, :, HD], 1.0)
        Vps.append(Vp_h)

    # ---- per-batch pipeline ---------------------------------------
    # Engines execute their queues in issue order, so cross-batch overlap
    # must exist in the issued stream: batch b's ACT-bound MHA head chunks
    # are interleaved with batch b+1's PE-bound P1-P5 chunks below.
    def _front(b):
        p = b % 2
        # P1: projections (k_i, q_j feature-major; v_i token-major)
        z_it, z_it8, z_jt8 = xs[b]

        k_iT = acts.tile([128, KT, N], F8, tag=f"kA{p}")
        q_jT = acts.tile([128, KT, N], F8, tag=f"qF{p}")
        for dst, wsrc, wco, xsrc in ((k_iT, wkvi_s, 0, z_it8),
                                     (q_jT, wqj_s, 0, z_jt8)):
            for mt in range(KT):
                for nh in range(2):
                    ps = pp_mm.tile([128, 512], F32, tag="mm", bufs=3)
                    dr_acc(nc, ps[:, :384],
                           [(wsrc[:, 2 * kp:2 * kp + 2,
                                  wco + mt * 128:wco + (mt + 1) * 128],
                             xsrc[:, 2 * kp:2 * kp + 2,
                                  nh * 384:(nh + 1) * 384])
                            for kp in range(KP)])
                    dcopy(nc, mt + nh,
                          dst[:, mt, nh * 384:(nh + 1) * 384], ps[:, :384])
                yield
        v_i = acts.tile([128, NT, F], F16, tag=f"v_i{p}")
        for nt in range(NT):
            ps = pp_mm.tile([128, 512], F32, tag="mm", bufs=3)
            dr_acc(nc, ps[:, :],
                   [(z_it8[:, 2 * kp:2 * kp + 2, nt * 128:(nt + 1) * 128],
                     wkvi_s[:, 2 * kp:2 * kp + 2, F:2 * F])
                    for kp in range(KP)])
            nc.scalar.copy(v_i[:, nt, :], ps[:, :])
            if nt % 2 == 1:
                yield

        # P2: a_ib attention (softmax over kv = N tokens, free axis)
        ctm = smalls.tile([128, NBT, F], F16, tag=f"ctm{p}")
        ctm8 = smalls.tile([128, NBT, F], F8, tag=f"ctm8{p}")
        cfm = smalls.tile([128, KT, NB], F8, tag=f"cfm{p}")
        A_ibT = smalls.tile([128, NT, NB], F16, tag=f"aib{p}")
        Eib = smalls.tile([128, NBT, N], F16, tag=f"esm{p}")
        for mt in range(NBT):
            st = pp_st.tile([128, 1024], F32, tag="st", bufs=2)
            for nh in range(2):
                dr_acc(nc, st[:, nh * 512:nh * 512 + 384],
                       [(q_bT[:, 2 * kp:2 * kp + 2, mt * 128:(mt + 1) * 128],
                         k_iT[:, 2 * kp:2 * kp + 2,
                              nh * 384:(nh + 1) * 384]) for kp in range(KP)])
            srs = smalls.tile([128, 2], F32, tag=f"srs{p}", bufs=4)
            E = Eib[:, mt]
            nc.scalar.activation(
                out=E[:, :].rearrange("q (c x) -> q c x", c=2),
                in_=st[:, :].rearrange("q (c x) -> q c x", c=2)[:, :, 0:384],
                func=AF.Exp, scale=SCALE, accum_out=srs[:, 0:1])
            nc.vector.reciprocal(srs[:, 1:2], srs[:, 0:1])
            nc.gpsimd.tensor_scalar_mul(E[:, :], E[:, :], srs[:, 1:2])
            # transpose E rows into A_ibT columns: 6 blocks
            pt = pp_mm.tile([128, 1024], F16, tag="mm", bufs=3, name="tr_ib")
            for j in range(NT):
                nc.tensor.transpose(pt[:, j * 128:(j + 1) * 128],
                                    E[:, j * 128:(j + 1) * 128], ident[:, :])
            nc.vector.tensor_copy(
                A_ibT[:, :, mt * 128:(mt + 1) * 128],
                pt[:, :NT * 128].rearrange("q (n x) -> q n x", x=128))
            yield
        for mt in range(NBT):
            ps = pp_mm.tile([128, 512], F32, tag="mm", bufs=3)
            mm_acc(nc, ps[:, :],
                   [(A_ibT[:, nt, mt * 128:(mt + 1) * 128], v_i[:, nt, :])
                    for nt in range(NT)])
            nc.vector.tensor_copy(ctm[:, mt, :], ps[:, :])
            nc.gpsimd.tensor_copy(ctm8[:, mt, :], ctm[:, mt, :])
        # ctx feature-major via transposes of ctx_tm (8 blocks, one bank)
        pt = pp_mm.tile([128, 1024], F16, tag="mm", bufs=3, name="tr_cf")
        for ft in range(KT):
            for mt in range(NBT):
                nc.tensor.transpose(
                    pt[:, (ft * NBT + mt) * 128:(ft * NBT + mt + 1) * 128],
                    ctm[:, mt, ft * 128:(ft + 1) * 128], ident[:, :])
        nc.vector.tensor_copy(cfm[:, :, :].rearrange("q a x -> q (a x)"),
                              pt[:, :])
        yield

        # P3: the two [N, NB] attentions (softmax over NB free axis)
        A_upT = smalls.tile([128, NBT, N], F8, tag=f"aup{p}")
        yield from _small_attention(nc, smalls, pp_mm, pp_st, ident,
                                    lhs=z_it8, rhs_fm=cfm, A_T=A_upT, p=p)
        A_bjT = smalls.tile([128, NBT, N], F8, tag=f"abj{p}")
        yield from _small_attention(nc, smalls, pp_mm, pp_st, ident,
                                    lhs=q_jT, rhs_fm=k_bT, A_T=A_bjT, p=p)

        # P4: a_ij feature-major; psum holds 2*A_SC*a_ij, drained with
        # 1/(2*A_SC) to store plain a_ij in fp8; reuses k_iT's slot
        aijT = acts.tile([128, KT, N], F8, tag=f"kA{p}")
        for ft in range(KT):
            for nh in range(2):
                ps = pp_mm.tile([128, 512], F32, tag="mm", bufs=3)
                pairs = [(ctm8[:, 0:2, ft * 128:(ft + 1) * 128],
                          A_upT[:, 0:2, nh * 384:(nh + 1) * 384]),
                         (v_b[:, 0:2, ft * 128:(ft + 1) * 128],
                          A_bjT[:, 0:2, nh * 384:(nh + 1) * 384])]
                dr_acc(nc, ps[:, :384], pairs)
                dscale(nc, ft + nh,
                       aijT[:, ft, nh * 384:(nh + 1) * 384], ps[:, :384],
                       1.0 / (2.0 * A_SC))
            yield

        # P5: gated fusion; f_T has its own fp16 slot
        f_T = acts.tile([128, KT, N], F16, tag=f"fT{p}")
        for ft in range(KT):
            st = pp_st.tile([128, 1024], F32, tag="st", bufs=2)
            for nh in range(2):
                pairs = [(wf_s[:, 2 * kp:2 * kp + 2, ft * 128:(ft + 1) * 128],
                          aijT[:, 2 * kp:2 * kp + 2, nh * 384:(nh + 1) * 384])
                         for kp in range(KP)]
                pairs += [(wf_s[:, KT + 2 * kp:KT + 2 * kp + 2,
                                ft * 128:(ft + 1) * 128],
                           z_jt8[:, 2 * kp:2 * kp + 2, nh * 384:(nh + 1) * 384])
                          for kp in range(KP)]
                dr_acc(nc, st[:, nh * 512:nh * 512 + 384], pairs)
            nc.scalar.activation(
                out=f_T[:, ft, :].rearrange("q (c x) -> q c x", c=2),
                in_=st[:, :].rearrange("q (c x) -> q c x", c=2)[:, :, 0:384],
                func=AF.Sigmoid, bias=bf_c[:, ft:ft + 1], scale=1.0)
            if ft % 2 == 1:
                yield
        h_T = acts.tile([128, KT, N], F16, tag=f"h{p}")
        for ft in range(KT):
            utmp = smalls.tile([128, N], F16, tag=f"utmp{p}", bufs=2)
            for nh in range(2):
                ps = pp_mm.tile([128, 512], F32, tag="mm", bufs=3)
                dr_acc(nc, ps[:, :384],
                       [(wm_s[:, 2 * kp:2 * kp + 2, ft * 128:(ft + 1) * 128],
                         aijT[:, 2 * kp:2 * kp + 2, nh * 384:(nh + 1) * 384])
                        for kp in range(KP)])
                nc.vector.scalar_tensor_tensor(
                    out=utmp[:, nh * 384:(nh + 1) * 384], in0=ps[:, :384],
                    scalar=bm_c[:, ft:ft + 1],
                    in1=f_T[:, ft, nh * 384:(nh + 1) * 384],
                    op0=ALU.add, op1=ALU.mult)
            nc.vector.tensor_tensor(out=utmp[:, :], in0=utmp[:, :],
                                    in1=z_it[:, ft, :], op=ALU.add)
            nc.vector.tensor_scalar_max(h_T[:, ft, :], utmp[:, :], 0.0)
            if ft % 2 == 1:
                yield
        state[b] = h_T

    state = {}

    def _qkv(b):
        h_T = state[b]
        p = b % 2
        # P6a: Q/K/V projections (per-parity tags so batch b+1 can project
        # while batch b is still inside its MHA heads)
        Q_T = acts.tile([128, KT, N], F16, tag=f"QT{p}")
        K_T = acts.tile([128, KT, N], F16, tag=f"KT{p}")
        for dst, co in ((Q_T, 0), (K_T, F)):
            for mt in range(KT):
                for nh in range(2):
                    ps = pp_mm.tile([128, 512], F32, tag="mm", bufs=3)
                    mm_acc(nc, ps[:, :384],
                           [(wqkv_s[:, k, co + mt * 128:co + (mt + 1) * 128],
                             h_T[:, k, nh * 384:(nh + 1) * 384])
                            for k in range(KT)])
                    dcopy(nc, mt + nh,
                          dst[:, mt, nh * 384:(nh + 1) * 384], ps[:, :384])
                yield
        Vp_h = Vps[p]
        for nt in range(NT):
            ps = pp_mm.tile([128, 512], F32, tag="mm", bufs=3)
            mm_acc(nc, ps[:, :],
                   [(h_T[:, k, nt * 128:(nt + 1) * 128],
                     wqkv_s[:, k, 2 * F:3 * F]) for k in range(KT)])
            dcopy(nc, (nt + 1) if b == 0 else 0,
                  Vp_h[:, nt, :, 0:HD],
                  ps[:, :].rearrange("p (h c) -> p h c", c=HD))
            if nt % 2 == 1:
                yield
        state[b] = (Q_T, K_T)

    def _mha(b):
        Q_T, K_T = state[b]
        p = b % 2
        Vp_h = Vps[p]
        H_tm = acts.tile([128, NT, F], F16, tag="Htm")
        for h in range(NHEADS):
            po = 64 * (h % 2)
            kt = h // 2
            Qh = Q_T[po:po + 64, kt, :]
            Kh = K_T[po:po + 64, kt, :]
            # full-bank tile: per-partition stride must be a 2KB multiple
            # for partition-offset (DoubleRow half) PSUM writes
            hp = pp_h.tile([128, 512], F32, tag="hp", bufs=1, name="hp")
            # zero the bank once; every AV matmul accumulates (interleaved
            # qt groups can't each own a start=True: it marks the whole
            # 2KB zero region pending)
            nc.tensor.matmul(hp[:, 0:390], ones_h[0:1, :], zrow[0:1, :],
                             start=True, stop=False, skip_group_check=True)
            for kv in range(NT):
                st = pp_st.tile([128, 1024], F32, tag="st", bufs=2)
                nc.tensor.matmul(st[:, 0:512],
                                 Kh[:, kv * 128:(kv + 1) * 128],
                                 Qh[:, 0:512],
                                 start=True, stop=True)
                nc.tensor.matmul(st[:, 512:768],
                                 Kh[:, kv * 128:(kv + 1) * 128],
                                 Qh[:, 512:768],
                                 start=True, stop=True)
                et = smalls.tile([128, N], F16, tag="et", bufs=2)
                nc.scalar.activation(out=et[:, :], in_=st[:, 0:768],
                                     func=AF.Exp, scale=MHA_SCALE)
                for qt in range(NT):
                    nc.tensor.matmul(
                        hp[:, qt * 65:(qt + 1) * 65],
                        et[:, qt * 128:(qt + 1) * 128],
                        Vp_h[:, kv, h, :],
                        start=False, stop=(kv == NT - 1),
                        skip_group_check=True)
            # single raw drain frees the hp bank for the next head
            hp_q = hp[:, 0:390].rearrange("p (q c) -> p q c", c=65)
            hr = smalls.tile([128, NT, HD + 1], F16, tag="hraw", bufs=2,
                             name="hr")
            nc.vector.tensor_copy(hr[:, :, :], hp_q[:, :, :])
            rt = smalls.tile([128, NT], F32, tag="rt", bufs=2)
            nc.vector.reciprocal(rt[:, :], hr[:, :, HD])
            for qt in range(NT):
                nc.gpsimd.tensor_scalar_mul(
                    H_tm[:, qt, h * 64:(h + 1) * 64],
                    hr[:, qt, 0:HD], rt[:, qt:qt + 1])
            # H_T transposes for feature-tile ft become ready after heads
            # 2ft and 2ft+1
            if h % 2 == 1:
                ft = h // 2
                if h == 1:
                    H_T = acts.tile([128, KT, N], F16, tag=f"HT{p}")
                    state[b] = ("HT", H_T)
                pt = pp_mm.tile([128, 1024], F16, tag="mm", bufs=3,
                                name="tr_h")
                for nt in range(NT):
                    nc.tensor.transpose(pt[:, nt * 128:(nt + 1) * 128],
                                        H_tm[:, nt, ft * 128:(ft + 1) * 128],
                                        ident[:, :])
                nc.vector.tensor_copy(H_T[:, ft, :], pt[:, :NT * 128])
            yield

    def _proj(b):
        _, H_T = state[b]
        for nt in range(NT):
            ps = pp_mm.tile([128, 512], F32, tag="mm", bufs=3)
            mm_acc(nc, ps[:, :],
                   [(H_T[:, k, nt * 128:(nt + 1) * 128], wproj_s[:, k, :])
                    for k in range(KT)])
            osb = smalls.tile([128, F], F16, tag="osb", bufs=2)
            nc.vector.tensor_tensor(out=osb[:, :], in0=ps[:, :],
                                    in1=bproj_s[:, :], op=ALU.add)
            dma_eng = nc.sync if nt % 2 == 0 else nc.scalar
            dma_eng.dma_start(out=out[b, nt * 128:(nt + 1) * 128, :],
                              in_=osb[:, :])
            yield

    # drive: batch-0 front interleaved 2:1 with batch-1 front; qkv0; then
    # mha0 heads interleaved with the rest of f1; once f1 finishes, qkv1
    # (per-parity Q/K/V tags make it independent of mha0) fills mha0's
    # exp-bound PE gaps; finally mha1 runs with proj0 as PE companionship.
    _SENT = object()

    def _step(g):
        return next(g, _SENT) is not _SENT

    f0 = _front(0)
    f1 = _front(1)
    done_f = False
    while True:
        if not _step(f0):
            break
        if not _step(f0):
            break
        if not done_f and not _step(f1):
            done_f = True
    for _ in _qkv(0):
        if not done_f and not _step(f1):
            done_f = True
    m0 = _mha(0)
    q1 = None
    done_m = done_q1 = False
    while not done_m:
        if not _step(m0):
            done_m = True
        for _ in range(3):
            if not done_f:
                done_f = _step(f1) is False
        if done_f and q1 is None:
            q1 = _qkv(1)
        if q1 is not None and not done_q1:
            for _ in range(2):
                if not _step(q1):
                    done_q1 = True
    while not done_f:
        done_f = _step(f1) is False
    if q1 is None:
        q1 = _qkv(1)
    while not done_q1:
        done_q1 = _step(q1) is False
    m1 = _mha(1)
    p0 = _proj(0)
    done_m1 = done_p0 = False
    while not done_m1:
        if not _step(m1):
            done_m1 = True
        if not done_p0:
            done_p0 = _step(p0) is False
    while not done_p0:
        done_p0 = _step(p0) is False
    for _ in _proj(1):
        pass


# ---------------------------------------------------------------------------
# Host-side wrapper
N_CORES = 8
_nc_cache = {}


def _get_nc(repeat=1):
    if repeat not in _nc_cache:
        nc = bass.Bass("TRN2", num_devices=N_CORES)
        build(nc, repeat=repeat)
        _split_waits(nc)
        _nc_cache[repeat] = nc
    return _nc_cache[repeat]


def _host_prep_shared(inputs):
    import ml_dtypes
    f8 = ml_dtypes.float8_e4m3

    def c8(a):
        return np.ascontiguousarray(
            np.clip(np.asarray(a, np.float32), -240.0, 240.0).astype(f8))

    def c(a, dt=np.float16):
        return np.ascontiguousarray(np.asarray(a, np.float32).astype(dt))

    return {
        "zbt8": c8(np.asarray(inputs["z_b"]).T),
        "wkv_i8": c8(np.asarray(inputs["Wqkv_i"])[:, F:]),
        "wq_j8": c8(np.asarray(inputs["Wqkv_j"])[:, :F]),
        "wqkv_b8": c8(inputs["Wqkv_b"]),
        "w_f8": c8(inputs["W_f"]),
        "w_m8": c8(inputs["W_m"]),
        "w_qkv": c(inputs["W_QKV"]),
        "w_proj": c(inputs["W_proj"]),
        "b_f": np.ascontiguousarray(np.asarray(inputs["b_f"], np.float32)),
        "b_m": np.ascontiguousarray(np.asarray(inputs["b_m"], np.float32)),
        "bproj_rep": np.ascontiguousarray(
            np.tile(np.asarray(inputs["b_proj"], np.float32).reshape(1, F),
                    (128, 1))),
    }


def make_in_maps(inputs):
    import ml_dtypes
    f8 = ml_dtypes.float8_e4m3
    x1 = np.asarray(inputs["x_1"], np.float32)
    x2 = np.asarray(inputs["x_2"], np.float32)
    B = x1.shape[0]
    assert B == N_CORES * B_LOC, (B, N_CORES, B_LOC)
    shared = _host_prep_shared(inputs)
    in_maps = []
    for c in range(N_CORES):
        sl = slice(c * B_LOC, (c + 1) * B_LOC)
        m = dict(shared)
        x1t = x1[sl].transpose(0, 2, 1)
        x2t = x2[sl].transpose(0, 2, 1)
        m["x1t"] = np.ascontiguousarray(x1t.astype(np.float16))
        m["x1t8"] = np.ascontiguousarray(
            np.clip(x1t, -240.0, 240.0).astype(f8))
        m["x2t8"] = np.ascontiguousarray(
            np.clip(x2t, -240.0, 240.0).astype(f8))
        in_maps.append(m)
    return in_maps


def kernel(**inputs) -> np.ndarray:
    nc = _get_nc(repeat=1)
    in_maps = make_in_maps(inputs)
    res = run_bass_kernel_spmd(nc, in_maps, list(range(N_CORES)))
    out = np.concatenate([np.asarray(r["out"]) for r in res.results], axis=0)
    return np.ascontiguousarray(out.astype(np.float32))


# revision 15
# speedup vs baseline: 1.7991x; 1.7991x over previous
"""Trainium2 Bass kernel for nn_CrossModalGatedBottleneckAttention (v3).

Contract: kernel(**inputs) takes the FULL unsharded inputs (as produced by
the problem's setup_inputs) and returns the full [16, 768, 512] output.

Strategy: data parallelism over batch B=16 across 8 NeuronCores (2 batches
per core). v3 over v2: fp8(e4m3)+DoubleRow matmuls for the error-tolerant
layers (P1 projections, bottleneck attention logits, av_up/av_bj, gated
fusion f/m, prologue) — 2x PE throughput on those. The error-sensitive
tail (QKV, MHA, out-proj) and av_ib stay fp16. A_up/A_bj attention weights
are stored x16 in fp8 to stay out of e4m3 subnormals; the 1/32 (incl. the
0.5 a_ij average) is folded into the P4 PSUM drain.
"""
import sys as _sys
for _p in ("/opt/trn_rl_repo",):
    if _p not in _sys.path:
        _sys.path.insert(0, _p)

import numpy as np
import concourse.bass as bass
import concourse.mybir as mybir
import concourse.tile as tile
from concourse.bass_utils import run_bass_kernel_spmd
from concourse.masks import make_identity

# ---------------------------------------------------------------------------
# Workaround for walrus sync-wait encoding limits: several instruction
# encodings in this neuronxcc build reject more than one sem-wait per
# instruction ("Too many sync wait commands"). After Tile scheduling, move
# all but one wait of each instruction onto same-engine NoOps inserted just
# before it. An engine blocks on each wait in order, so semantics are
# preserved.
_wsplit_ctr = [0]


def _split_waits(nc, max_waits=1):
    n_split = 0
    for f in nc.m.functions:
        for blk in f.blocks:
            insts = blk.instructions
            new_list = []
            changed = False
            for inst in insts:
                si = inst.sync_info
                if si is not None and si.on_wait and len(si.on_wait) > max_waits:
                    waits = list(si.on_wait)
                    extra, keep = waits[:-max_waits], waits[-max_waits:]
                    for w in extra:
                        _wsplit_ctr[0] += 1
                        nop = mybir.InstNoOp(
                            name=f"I-wsplit-{_wsplit_ctr[0]}", ins=[], outs=[])
                        nop.engine = inst.engine
                        nop.sync_info = mybir.SyncInfo(on_wait=[w], on_update=[])
                        new_list.append(nop)
                        n_split += 1
                    inst.sync_info = mybir.SyncInfo(
                        on_wait=keep, on_update=list(si.on_update or []))
                    changed = True
                new_list.append(inst)
            if changed:
                insts.clear()
                insts.extend(new_list)
    return n_split


# ---------------------------------------------------------------------------
F32 = mybir.dt.float32
F16 = mybir.dt.float16
F8 = mybir.dt.float8e4
AF = mybir.ActivationFunctionType
ALU = mybir.AluOpType
DR = mybir.MatmulPerfMode.DoubleRow

F = 512
N = 768
NB = 256
NHEADS = 8
HD = 64
KT = F // 128            # 4 k-tiles over feature dim
KP = KT // 2             # 2 DoubleRow k-pair tiles
NT = N // 128            # 6 tiles over tokens
NBT = NB // 128          # 2 tiles over bottleneck tokens
SCALE = float(F) ** -0.5
MHA_SCALE = float(HD) ** -0.5
B_LOC = 2                # batches per core
A_SC = 16.0              # fp8 storage scale for A_up/A_bj attention weights


def mm_acc(nc, psum_ap, pairs):
    n = len(pairs)
    for i, (l, r) in enumerate(pairs):
        nc.tensor.matmul(psum_ap, l, r, start=(i == 0), stop=(i == n - 1))


def dcopy(nc, i, out, in_):
    """PSUM drain copy, alternating DVE / ACT (GpSimd cannot touch PSUM)."""
    if i % 2 == 0:
        nc.vector.tensor_copy(out, in_)
    else:
        nc.scalar.copy(out, in_)


def dscale(nc, i, out, in_, c):
    """PSUM drain with scalar multiply, alternating DVE / ACT."""
    if i % 2 == 0:
        nc.vector.tensor_scalar_mul(out, in_, c)
    else:
        nc.scalar.mul(out, in_, c)


def dr_acc(nc, ps_cols, pairs):
    """fp8 DoubleRow accumulation into a [128, n] PSUM region.

    pairs: list of (lhsT [128, 2, 128], rhs [128, 2, n]) fp8 APs; each
    matmul contracts both k-planes (256 rows) at 2 fp8 weights per PE cell.
    """
    npair = len(pairs)
    for i, (l, r) in enumerate(pairs):
        nc.tensor.matmul(ps_cols, l, r, start=(i == 0),
                         stop=(i == npair - 1), perf_mode=DR)


def build(nc: bass.Bass, repeat: int = 1):
    dram = {}

    def din(name, shape, dt=F16):
        dram[name] = nc.declare_dram_parameter(name, list(shape), dt,
                                               isOutput=False)
        return dram[name]

    for name, shape, dt in [
            ("x1t", [B_LOC, F, N], F16),
            ("x1t8", [B_LOC, F, N], F8), ("x2t8", [B_LOC, F, N], F8),
            ("zbt8", [F, NB], F8),
            ("wkv_i8", [F, 2 * F], F8), ("wq_j8", [F, F], F8),
            ("wqkv_b8", [F, 3 * F], F8),
            ("w_f8", [2 * F, F], F8), ("w_m8", [F, F], F8),
            ("w_qkv", [F, 3 * F], F16), ("w_proj", [F, F], F16)]:
        din(name, shape, dt)
    din("b_f", [F], F32)
    din("b_m", [F], F32)
    din("bproj_rep", [128, F], F32)
    out = nc.declare_dram_parameter("out", [B_LOC, N, F], F16, isOutput=True)

    with tile.TileContext(nc) as tc:
        if repeat == 1:
            _body(nc, tc, dram, out)
        else:
            with tc.For_i(0, repeat, 1,
                          hint_engines=(mybir.EngineType.PE,
                                        mybir.EngineType.DVE),
                          staggered_reset=True):
                _body(nc, tc, dram, out)
    return nc


def _wview(ap):
    # [Fin, Fout] dram -> [128, Fin//128, Fout] partition-tiled view
    return ap[:, :].rearrange("(k p) o -> p k o", p=128)


def _xview(ap):
    # [F, N] dram -> [128, KT, N]
    return ap.rearrange("(k p) n -> p k n", p=128)


def _body(nc, tc, dram, out):
    import contextlib
    with contextlib.ExitStack() as ctx:
        consts = ctx.enter_context(tc.tile_pool(name="consts", bufs=1))
        wts = ctx.enter_context(tc.tile_pool(name="wts", bufs=1))
        acts = ctx.enter_context(tc.tile_pool(name="acts", bufs=1))
        smalls = ctx.enter_context(tc.tile_pool(name="smalls", bufs=1))
        pp_st = ctx.enter_context(tc.tile_pool(name="pp_st", bufs=1, space="PSUM"))
        pp_mm = ctx.enter_context(tc.tile_pool(name="pp_mm", bufs=1, space="PSUM"))
        pp_h = ctx.enter_context(tc.tile_pool(name="pp_h", bufs=1, space="PSUM"))
        _emit(nc, tc, dram, out, consts, wts, acts, smalls, pp_st, pp_mm, pp_h)


def _small_attention(nc, smalls, pp_mm, pp_st, ident, lhs, rhs_fm, A_T, p):
    """A = A_SC * softmax(lhs.T @ rhs_fm * SCALE, axis=-1) written to A_T
    transposed [NB, N] fp8. lhs, rhs_fm fp8. Generator: yields at chunk
    boundaries."""
    E = smalls.tile([128, NT, NB], F16, tag=f"esm{p}")
    for nt in range(NT):
        ps = pp_mm.tile([128, 512], F32, tag="mm", bufs=3)
        dr_acc(nc, ps[:, :NB],
               [(lhs[:, 2 * kp:2 * kp + 2, nt * 128:(nt + 1) * 128],
                 rhs_fm[:, 2 * kp:2 * kp + 2, :]) for kp in range(KP)])
        srs = smalls.tile([128, 2], F32, tag=f"srs{p}", bufs=4)
        nc.scalar.activation(out=E[:, nt, :], in_=ps[:, :NB], func=AF.Exp,
                             scale=SCALE, accum_out=srs[:, 0:1])
        nc.vector.reciprocal(srs[:, 1:2], srs[:, 0:1])
        nc.vector.tensor_scalar(out=E[:, nt, :], in0=E[:, nt, :],
                                scalar1=srs[:, 1:2], scalar2=A_SC,
                                op0=ALU.mult, op1=ALU.mult)
        if nt % 3 == 2:
            yield
    # transposes: per mt, 6 [128,128] blocks -> one PSUM bank -> wide copy
    for mt in range(NBT):
        pt = pp_mm.tile([128, 1024], F16, tag="mm", bufs=3, name="tr_sm")
        for j in range(NT):
            nc.tensor.transpose(pt[:, j * 128:(j + 1) * 128],
                                E[:, j, mt * 128:(mt + 1) * 128], ident[:, :])
        nc.vector.tensor_copy(A_T[:, mt, :], pt[:, :NT * 128])
        yield


def _emit(nc, tc, dram, out, consts, wts, acts, smalls, pp_st, pp_mm, pp_h):
    # ---- constants -------------------------------------------------
    ident = consts.tile([128, 128], F16)
    make_identity(nc, ident)
    bf_c = consts.tile([128, KT], F32)
    nc.sync.dma_start(out=bf_c, in_=dram["b_f"][:].rearrange("(k p) -> p k", p=128))
    bm_c = consts.tile([128, KT], F32)
    nc.sync.dma_start(out=bm_c, in_=dram["b_m"][:].rearrange("(k p) -> p k", p=128))
    bproj_s = consts.tile([128, F], F32)
    nc.sync.dma_start(out=bproj_s, in_=dram["bproj_rep"][:, :])
    zrow = consts.tile([1, 390], F16)
    nc.vector.memset(zrow[:, :], 0.0)
    ones_h = consts.tile([1, 128], F16)
    nc.vector.memset(ones_h[:, :], 1.0)

    # ---- prologue: bottleneck projections (batch independent) ------
    # DMA issue order = queue order: first the tensors feeding the first PE
    # work (zbt, wqkv_b), then both batches' inputs so batch 1's loads do
    # not queue behind batch 0's output stores, then weights by first use.
    # split the wqkv_b load so the first prologue matmuls (q_b, needs cols
    # 0:F) can start before the k/v columns arrive
    wqkvb_s = wts.tile([128, KT, 3 * F], F8, tag="wqkvb")
    nc.sync.dma_start(out=wqkvb_s[:, :, 0:F], in_=_wview(dram["wqkv_b8"])[:, :, 0:F])
    zbt_s = smalls.tile([128, KT, NB], F8, tag="zbt")
    nc.sync.dma_start(out=zbt_s, in_=_xview(dram["zbt8"][:, :]))
    nc.sync.dma_start(out=wqkvb_s[:, :, F:3 * F],
                      in_=_wview(dram["wqkv_b8"])[:, :, F:3 * F])
    xs = []
    for b in range(B_LOC):
        z_it = acts.tile([128, KT, N], F16, tag=f"z_it{b % 2}", name="z_it")
        z_it8 = acts.tile([128, KT, N], F8, tag=f"z_it8{b % 2}", name="z_it8")
        z_jt8 = acts.tile([128, KT, N], F8, tag=f"z_jt8{b % 2}", name="z_jt8")
        xs.append((z_it, z_it8, z_jt8))
    nc.sync.dma_start(out=xs[0][1], in_=_xview(dram["x1t8"][0]))
    nc.sync.dma_start(out=xs[0][2], in_=_xview(dram["x2t8"][0]))
    wkvi_s = wts.tile([128, KT, 2 * F], F8, tag="wkvi")
    nc.sync.dma_start(out=wkvi_s, in_=_wview(dram["wkv_i8"]))
    wqj_s = wts.tile([128, KT, F], F8, tag="wqj")
    nc.sync.dma_start(out=wqj_s, in_=_wview(dram["wq_j8"]))
    nc.sync.dma_start(out=xs[0][0], in_=_xview(dram["x1t"][0]))
    nc.sync.dma_start(out=xs[1][1], in_=_xview(dram["x1t8"][1]))
    nc.sync.dma_start(out=xs[1][2], in_=_xview(dram["x2t8"][1]))
    nc.sync.dma_start(out=xs[1][0], in_=_xview(dram["x1t"][1]))
    wf_s = wts.tile([128, 2 * KT, F], F8, tag="wf")
    nc.sync.dma_start(out=wf_s, in_=_wview(dram["w_f8"]))
    wm_s = wts.tile([128, KT, F], F8, tag="wm")
    nc.sync.dma_start(out=wm_s, in_=_wview(dram["w_m8"]))
    wqkv_s = wts.tile([128, KT, 3 * F], F16, tag="wqkv")
    nc.sync.dma_start(out=wqkv_s, in_=_wview(dram["w_qkv"]))
    wproj_s = wts.tile([128, KT, F], F16, tag="wproj")
    nc.sync.dma_start(out=wproj_s, in_=_wview(dram["w_proj"]))

    q_bT = consts.tile([128, KT, NB], F8)
    k_bT = consts.tile([128, KT, NB], F8)
    for dst, co in ((q_bT, 0), (k_bT, F)):
        for mt in range(KT):
            ps = pp_mm.tile([128, 512], F32, tag="mm", bufs=3)
            dr_acc(nc, ps[:, :NB],
                   [(wqkvb_s[:, 2 * kp:2 * kp + 2,
                             co + mt * 128:co + (mt + 1) * 128],
                     zbt_s[:, 2 * kp:2 * kp + 2, :]) for kp in range(KP)])
            dscale(nc, mt, dst[:, mt, :], ps[:, :NB], 0.2)
    v_b = consts.tile([128, NBT, F], F8)
    for mt in range(NBT):
        ps = pp_mm.tile([128, 512], F32, tag="mm", bufs=3)
        dr_acc(nc, ps[:, :],
               [(zbt_s[:, 2 * kp:2 * kp + 2, mt * 128:(mt + 1) * 128],
                 wqkvb_s[:, 2 * kp:2 * kp + 2, 2 * F:3 * F])
                for kp in range(KP)])
        dscale(nc, mt + 1, v_b[:, mt, :], ps[:, :], 0.2)

    # MHA V layout (per batch parity): per token-tile, 8 heads x (64 V cols
    # + 1 ones col for the softmax denominator)
    Vps = []
    for p in range(2):
        V_plus = smalls.tile([128, NT, NHEADS * (HD + 1)], F16, tag=f"vp{p}",
                             name="V_plus")
        Vp_h = V_plus.rearrange("p n (h c) -> p n h c", c=HD + 1)
        nc.vector.memset(Vp_h[:, :, :, HD], 1.0)
        Vps.append(Vp_h)

    # ---- per-batch pipeline ---------------------------------------
    # Engines execute their queues in issue order, so cross-batch overlap
    # must exist in the issued stream: batch b's ACT-bound MHA head chunks
    # are interleaved with batch b+1's PE-bound P1-P5 chunks below.
    def _front(b):
        p = b % 2
        # P1: projections (k_i, q_j feature-major; v_i token-major)
        z_it, z_it8, z_jt8 = xs[b]

        k_iT = acts.tile([128, KT, N], F8, tag=f"kA{p}")
        q_jT = acts.tile([128, KT, N], F8, tag=f"qF{p}")
        for dst, wsrc, wco, xsrc in ((k_iT, wkvi_s, 0, z_it8),
                                     (q_jT, wqj_s, 0, z_jt8)):
            for mt in range(KT):
                for nh in range(2):
                    ps = pp_mm.tile([128, 512], F32, tag="mm", bufs=3)
                    dr_acc(nc, ps[:, :384],
                           [(wsrc[:, 2 * kp:2 * kp + 2,
                                  wco + mt * 128:wco + (mt + 1) * 128],
                             xsrc[:, 2 * kp:2 * kp + 2,
                                  nh * 384:(nh + 1) * 384])
                            for kp in range(KP)])
                    dcopy(nc, mt + nh,
                          dst[:, mt, nh * 384:(nh + 1) * 384], ps[:, :384])
                yield
        v_i = acts.tile([128, NT, F], F16, tag=f"v_i{p}")
        for nt in range(NT):
            ps = pp_mm.tile([128, 512], F32, tag="mm", bufs=3)
            dr_acc(nc, ps[:, :],
                   [(z_it8[:, 2 * kp:2 * kp + 2, nt * 128:(nt + 1) * 128],
                     wkvi_s[:, 2 * kp:2 * kp + 2, F:2 * F])
                    for kp in range(KP)])
            nc.scalar.copy(v_i[:, nt, :], ps[:, :])
            if nt % 2 == 1:
                yield

        # P2: a_ib attention (softmax over kv = N tokens, free axis)
        ctm = smalls.tile([128, NBT, F], F16, tag=f"ctm{p}")
        ctm8 = smalls.tile([128, NBT, F], F8, tag=f"ctm8{p}")
        cfm = smalls.tile([128, KT, NB], F8, tag=f"cfm{p}")
        A_ibT = smalls.tile([128, NT, NB], F16, tag=f"aib{p}")
        Eib = smalls.tile([128, NBT, N], F16, tag=f"esm{p}")
        for mt in range(NBT):
            st = pp_st.tile([128, 1024], F32, tag="st", bufs=2)
            for nh in range(2):
                dr_acc(nc, st[:, nh * 512:nh * 512 + 384],
                       [(q_bT[:, 2 * kp:2 * kp + 2, mt * 128:(mt + 1) * 128],
                         k_iT[:, 2 * kp:2 * kp + 2,
                              nh * 384:(nh + 1) * 384]) for kp in range(KP)])
            srs = smalls.tile([128, 2], F32, tag=f"srs{p}", bufs=4)
            E = Eib[:, mt]
            nc.scalar.activation(
                out=E[:, :].rearrange("q (c x) -> q c x", c=2),
                in_=st[:, :].rearrange("q (c x) -> q c x", c=2)[:, :, 0:384],
                func=AF.Exp, scale=SCALE, accum_out=srs[:, 0:1])
            nc.vector.reciprocal(srs[:, 1:2], srs[:, 0:1])
            nc.vector.tensor_scalar_mul(E[:, :], E[:, :], srs[:, 1:2])
            # transpose E rows into A_ibT columns: 6 blocks
            pt = pp_mm.tile([128, 1024], F16, tag="mm", bufs=3, name="tr_ib")
            for j in range(NT):
                nc.tensor.transpose(pt[:, j * 128:(j + 1) * 128],
                                    E[:, j * 128:(j + 1) * 128], ident[:, :])
            nc.vector.tensor_copy(
                A_ibT[:, :, mt * 128:(mt + 1) * 128],
                pt[:, :NT * 128].rearrange("q (n x) -> q n x", x=128))
            yield
        for mt in range(NBT):
            ps = pp_mm.tile([128, 512], F32, tag="mm", bufs=3)
            mm_acc(nc, ps[:, :],
                   [(A_ibT[:, nt, mt * 128:(mt + 1) * 128], v_i[:, nt, :])
                    for nt in range(NT)])
            nc.vector.tensor_copy(ctm[:, mt, :], ps[:, :])
            nc.vector.tensor_copy(ctm8[:, mt, :], ctm[:, mt, :])
        # ctx feature-major via transposes of ctx_tm (8 blocks, one bank)
        pt = pp_mm.tile([128, 1024], F16, tag="mm", bufs=3, name="tr_cf")
        for ft in range(KT):
            for mt in range(NBT):
                nc.tensor.transpose(
                    pt[:, (ft * NBT + mt) * 128:(ft * NBT + mt + 1) * 128],
                    ctm[:, mt, ft * 128:(ft + 1) * 128], ident[:, :])
        nc.vector.tensor_copy(cfm[:, :, :].rearrange("q a x -> q (a x)"),
                              pt[:, :])
        yield

        # P3: the two [N, NB] attentions (softmax over NB free axis)
        A_upT = smalls.tile([128, NBT, N], F8, tag=f"aup{p}")
        yield from _small_attention(nc, smalls, pp_mm, pp_st, ident,
                                    lhs=z_it8, rhs_fm=cfm, A_T=A_upT, p=p)
        A_bjT = smalls.tile([128, NBT, N], F8, tag=f"abj{p}")
        yield from _small_attention(nc, smalls, pp_mm, pp_st, ident,
                                    lhs=q_jT, rhs_fm=k_bT, A_T=A_bjT, p=p)

        # P4: a_ij feature-major; psum holds 2*A_SC*a_ij, drained with
        # 1/(2*A_SC) to store plain a_ij in fp8; reuses k_iT's slot
        aijT = acts.tile([128, KT, N], F8, tag=f"kA{p}")
        for ft in range(KT):
            for nh in range(2):
                ps = pp_mm.tile([128, 512], F32, tag="mm", bufs=3)
                pairs = [(ctm8[:, 0:2, ft * 128:(ft + 1) * 128],
                          A_upT[:, 0:2, nh * 384:(nh + 1) * 384]),
                         (v_b[:, 0:2, ft * 128:(ft + 1) * 128],
                          A_bjT[:, 0:2, nh * 384:(nh + 1) * 384])]
                dr_acc(nc, ps[:, :384], pairs)
                dscale(nc, ft + nh,
                       aijT[:, ft, nh * 384:(nh + 1) * 384], ps[:, :384],
                       1.0 / (2.0 * A_SC))
            yield

        # P5: gated fusion; f_T has its own fp16 slot
        f_T = acts.tile([128, KT, N], F16, tag=f"fT{p}")
        for ft in range(KT):
            st = pp_st.tile([128, 1024], F32, tag="st", bufs=2)
            for nh in range(2):
                pairs = [(wf_s[:, 2 * kp:2 * kp + 2, ft * 128:(ft + 1) * 128],
                          aijT[:, 2 * kp:2 * kp + 2, nh * 384:(nh + 1) * 384])
                         for kp in range(KP)]
                pairs += [(wf_s[:, KT + 2 * kp:KT + 2 * kp + 2,
                                ft * 128:(ft + 1) * 128],
                           z_jt8[:, 2 * kp:2 * kp + 2, nh * 384:(nh + 1) * 384])
                          for kp in range(KP)]
                dr_acc(nc, st[:, nh * 512:nh * 512 + 384], pairs)
            nc.scalar.activation(
                out=f_T[:, ft, :].rearrange("q (c x) -> q c x", c=2),
                in_=st[:, :].rearrange("q (c x) -> q c x", c=2)[:, :, 0:384],
                func=AF.Sigmoid, bias=bf_c[:, ft:ft + 1], scale=1.0)
            if ft % 2 == 1:
                yield
        h_T = acts.tile([128, KT, N], F16, tag=f"h{p}")
        for ft in range(KT):
            utmp = smalls.tile([128, N], F16, tag=f"utmp{p}", bufs=2)
            for nh in range(2):
                ps = pp_mm.tile([128, 512], F32, tag="mm", bufs=3)
                dr_acc(nc, ps[:, :384],
                       [(wm_s[:, 2 * kp:2 * kp + 2, ft * 128:(ft + 1) * 128],
                         aijT[:, 2 * kp:2 * kp + 2, nh * 384:(nh + 1) * 384])
                        for kp in range(KP)])
                nc.vector.scalar_tensor_tensor(
                    out=utmp[:, nh * 384:(nh + 1) * 384], in0=ps[:, :384],
                    scalar=bm_c[:, ft:ft + 1],
                    in1=f_T[:, ft, nh * 384:(nh + 1) * 384],
                    op0=ALU.add, op1=ALU.mult)
            nc.vector.tensor_tensor(out=utmp[:, :], in0=utmp[:, :],
                                    in1=z_it[:, ft, :], op=ALU.add)
            nc.vector.tensor_scalar_max(h_T[:, ft, :], utmp[:, :], 0.0)
            if ft % 2 == 1:
                yield
        state[b] = h_T

    state = {}

    def _qkv(b):
        h_T = state[b]
        p = b % 2
        # P6a: Q/K/V projections (per-parity tags so batch b+1 can project
        # while batch b is still inside its MHA heads)
        Q_T = acts.tile([128, KT, N], F16, tag=f"QT{p}")
        K_T = acts.tile([128, KT, N], F16, tag=f"KT{p}")
        for dst, co in ((Q_T, 0), (K_T, F)):
            for mt in range(KT):
                for nh in range(2):
                    ps = pp_mm.tile([128, 512], F32, tag="mm", bufs=3)
                    mm_acc(nc, ps[:, :384],
                           [(wqkv_s[:, k, co + mt * 128:co + (mt + 1) * 128],
                             h_T[:, k, nh * 384:(nh + 1) * 384])
                            for k in range(KT)])
                    dcopy(nc, mt + nh,
                          dst[:, mt, nh * 384:(nh + 1) * 384], ps[:, :384])
                yield
        Vp_h = Vps[p]
        for nt in range(NT):
            ps = pp_mm.tile([128, 512], F32, tag="mm", bufs=3)
            mm_acc(nc, ps[:, :],
                   [(h_T[:, k, nt * 128:(nt + 1) * 128],
                     wqkv_s[:, k, 2 * F:3 * F]) for k in range(KT)])
            dcopy(nc, (nt + 1) if b == 0 else 0,
                  Vp_h[:, nt, :, 0:HD],
                  ps[:, :].rearrange("p (h c) -> p h c", c=HD))
            if nt % 2 == 1:
                yield
        state[b] = (Q_T, K_T)

    def _mha(b):
        Q_T, K_T = state[b]
        p = b % 2
        Vp_h = Vps[p]
        H_tm = acts.tile([128, NT, F], F16, tag="Htm")
        for h in range(NHEADS):
            po = 64 * (h % 2)
            kt = h // 2
            Qh = Q_T[po:po + 64, kt, :]
            Kh = K_T[po:po + 64, kt, :]
            # full-bank tile: per-partition stride must be a 2KB multiple
            # for partition-offset (DoubleRow half) PSUM writes
            hp = pp_h.tile([128, 512], F32, tag="hp", bufs=1, name="hp")
            # zero the bank once; every AV matmul accumulates (interleaved
            # qt groups can't each own a start=True: it marks the whole
            # 2KB zero region pending)
            nc.tensor.matmul(hp[:, 0:390], ones_h[0:1, :], zrow[0:1, :],
                             start=True, stop=False, skip_group_check=True)
            for kv in range(NT):
                st = pp_st.tile([128, 1024], F32, tag="st", bufs=2)
                nc.tensor.matmul(st[:, 0:512],
                                 Kh[:, kv * 128:(kv + 1) * 128],
                                 Qh[:, 0:512],
                                 start=True, stop=True)
                nc.tensor.matmul(st[:, 512:768],
                                 Kh[:, kv * 128:(kv + 1) * 128],
                                 Qh[:, 512:768],
                                 start=True, stop=True)
                et = smalls.tile([128, N], F16, tag="et", bufs=2)
                nc.scalar.activation(out=et[:, :], in_=st[:, 0:768],
                                     func=AF.Exp, scale=MHA_SCALE)
                for qt in range(NT):
                    nc.tensor.matmul(
                        hp[:, qt * 65:(qt + 1) * 65],
                        et[:, qt * 128:(qt + 1) * 128],
                        Vp_h[:, kv, h, :],
                        start=False, stop=(kv == NT - 1),
                        skip_group_check=True)
            # single raw drain frees the hp bank for the next head
            hp_q = hp[:, 0:390].rearrange("p (q c) -> p q c", c=65)
            hr = smalls.tile([128, NT, HD + 1], F16, tag="hraw", bufs=2,
                             name="hr")
            nc.vector.tensor_copy(hr[:, :, :], hp_q[:, :, :])
            rt = smalls.tile([128, NT], F32, tag="rt", bufs=2)
            nc.vector.reciprocal(rt[:, :], hr[:, :, HD])
            for qt in range(NT):
                nc.vector.tensor_scalar_mul(
                    H_tm[:, qt, h * 64:(h + 1) * 64],
                    hr[:, qt, 0:HD], rt[:, qt:qt + 1])
            # H_T transposes for feature-tile ft become ready after heads
            # 2ft and 2ft+1
            if h % 2 == 1:
                ft = h // 2
                if h == 1:
                    H_T = acts.tile([128, KT, N], F16, tag=f"HT{p}")
                    state[b] = ("HT", H_T)
                pt = pp_mm.tile([128, 1024], F16, tag="mm", bufs=3,
                                name="tr_h")
                for nt in range(NT):
                    nc.tensor.transpose(pt[:, nt * 128:(nt + 1) * 128],
                                        H_tm[:, nt, ft * 128:(ft + 1) * 128],
                                        ident[:, :])
                nc.vector.tensor_copy(H_T[:, ft, :], pt[:, :NT * 128])
            yield

    def _proj(b):
        _, H_T = state[b]
        for nt in range(NT):
            ps = pp_mm.tile([128, 512], F32, tag="mm", bufs=3)
            mm_acc(nc, ps[:, :],
                   [(H_T[:, k, nt * 128:(nt + 1) * 128], wproj_s[:, k, :])
                    for k in range(KT)])
            osb = smalls.tile([128, F], F16, tag="osb", bufs=2)
            nc.vector.tensor_tensor(out=osb[:, :], in0=ps[:, :],
                                    in1=bproj_s[:, :], op=ALU.add)
            nc.sync.dma_start(out=out[b, nt * 128:(nt + 1) * 128, :],
                              in_=osb[:, :])
            yield

    # drive: batch-0 front interleaved 2:1 with batch-1 front; qkv0; then
    # mha0 heads interleaved with the rest of f1; once f1 finishes, qkv1
    # (per-parity Q/K/V tags make it independent of mha0) fills mha0's
    # exp-bound PE gaps; finally mha1 runs with proj0 as PE companionship.
    _SENT = object()

    def _step(g):
        return next(g, _SENT) is not _SENT

    f0 = _front(0)
    f1 = _front(1)
    done_f = False
    while True:
        if not _step(f0):
            break
        if not _step(f0):
            break
        if not done_f and not _step(f1):
            done_f = True
    for _ in _qkv(0):
        if not done_f and not _step(f1):
            done_f = True
    m0 = _mha(0)
    q1 = None
    done_m = done_q1 = False
    while not done_m:
        if not _step(m0):
            done_m = True
        for _ in range(3):
            if not done_f:
                done_f = _step(f1) is False
        if done_f and q1 is None:
            q1 = _qkv(1)
        if q1 is not None and not done_q1:
            for _ in range(2):
                if not _step(q1):
                    done_q1 = True
    while not done_f:
        done_f = _step(f1) is False
    if q1 is None:
        q1 = _qkv(1)
    while not done_q1:
        done_q1 = _step(q1) is False
    m1 = _mha(1)
    p0 = _proj(0)
    done_m1 = done_p0 = False
    while not done_m1:
        if not _step(m1):
            done_m1 = True
        if not done_p0:
            done_p0 = _step(p0) is False
    while not done_p0:
        done_p0 = _step(p0) is False
    for _ in _proj(1):
        pass


# ---------------------------------------------------------------------------
# Host-side wrapper
N_CORES = 8
_nc_cache = {}


def _get_nc(repeat=1):
    if repeat not in _nc_cache:
        nc = bass.Bass("TRN2", num_devices=N_CORES)
        build(nc, repeat=repeat)
        _split_waits(nc)
        _nc_cache[repeat] = nc
    return _nc_cache[repeat]


def _host_prep_shared(inputs):
    import ml_dtypes
    f8 = ml_dtypes.float8_e4m3

    def c8(a):
        return np.ascontiguousarray(
            np.clip(np.asarray(a, np.float32), -240.0, 240.0).astype(f8))

    def c(a, dt=np.float16):
        return np.ascontiguousarray(np.asarray(a, np.float32).astype(dt))

    return {
        "zbt8": c8(np.asarray(inputs["z_b"]).T),
        "wkv_i8": c8(np.asarray(inputs["Wqkv_i"])[:, F:]),
        "wq_j8": c8(np.asarray(inputs["Wqkv_j"])[:, :F]),
        "wqkv_b8": c8(inputs["Wqkv_b"]),
        "w_f8": c8(inputs["W_f"]),
        "w_m8": c8(inputs["W_m"]),
        "w_qkv": c(inputs["W_QKV"]),
        "w_proj": c(inputs["W_proj"]),
        "b_f": np.ascontiguousarray(np.asarray(inputs["b_f"], np.float32)),
        "b_m": np.ascontiguousarray(np.asarray(inputs["b_m"], np.float32)),
        "bproj_rep": np.ascontiguousarray(
            np.tile(np.asarray(inputs["b_proj"], np.float32).reshape(1, F),
                    (128, 1))),
    }


def make_in_maps(inputs):
    import ml_dtypes
    f8 = ml_dtypes.float8_e4m3
    x1 = np.asarray(inputs["x_1"], np.float32)
    x2 = np.asarray(inputs["x_2"], np.float32)
    B = x1.shape[0]
    assert B == N_CORES * B_LOC, (B, N_CORES, B_LOC)
    shared = _host_prep_shared(inputs)
    in_maps = []
    for c in range(N_CORES):
        sl = slice(c * B_LOC, (c + 1) * B_LOC)
        m = dict(shared)
        x1t = x1[sl].transpose(0, 2, 1)
        x2t = x2[sl].transpose(0, 2, 1)
        m["x1t"] = np.ascontiguousarray(x1t.astype(np.float16))
        m["x1t8"] = np.ascontiguousarray(
            np.clip(x1t, -240.0, 240.0).astype(f8))
        m["x2t8"] = np.ascontiguousarray(
            np.clip(x2t, -240.0, 240.0).astype(f8))
        in_maps.append(m)
    return in_maps


def kernel(**inputs) -> np.ndarray:
    nc = _get_nc(repeat=1)
    in_maps = make_in_maps(inputs)
    res = run_bass_kernel_spmd(nc, in_maps, list(range(N_CORES)))
    out = np.concatenate([np.asarray(r["out"]) for r in res.results], axis=0)
    return np.ascontiguousarray(out.astype(np.float32))
